# revision 29
# baseline (speedup 1.0000x reference)
"""GCN message-passing kernel for 8 TRN2 NeuronCores.

Strategy (graph/data parallel, dst-sharded):
  - Nodes sharded 6250/core. Per layer: each core computes its shard of the
    gather table H' (dense matmul + epilogue), an AllGather builds the full
    50000x128 fp16 table in HBM.
  - Edges (incl. self-loops) sorted by dst, grouped into 128-wide dst blocks.
    Per 128-edge tile: dma_gather fetches H'[src] rows (256B fp16), DVE
    is_equal builds the one-hot [edge, dst_local] tile, PE accumulates
    Onehot^T @ G into the block's PSUM (segment sum).
  - GCN norm dinv[src]*dinv[dst] is separable: src side folded into the
    table rows, dst side into the block epilogue. BN folded into W and B.
  - Epilogues: dense1: tbl = dinv*(x@W1A); gather1/2: z = dinv*relu(S*dinv+B)
    (the outer dinv pre-applies the next table's src factor); dense2/3 are
    then plain copies; gather3: y3 = relu(S*dinv+B) feeds the MLP head.
  - 4-layer MLP runs feature-major per core; output [8, 6272] f32 is
    transposed/trimmed on the host.
"""

import sys

sys.path.insert(0, "/opt/trn_rl_repo")

import numpy as np

import concourse.bacc as bacc
import concourse.mybir as mybir
import concourse.tile as tile
from concourse.bass_utils import run_bass_kernel_spmd

# Problem constants
N, E, IN, HC = 50000, 800000, 64, 96
FC1, FC2, FC3, OUT = 256, 128, 64, 8
EPS = 1e-5
NCORES = 8
NPER = N // NCORES          # 6250 nodes per core
BW = 128                    # dst-block width
NB = (NPER + BW - 1) // BW  # 49 blocks
NPAD = NB * BW              # 6272
TC = 8                      # tiles per gather chunk (1024 idxs; >=1920 hangs HW)
W = 30                      # lo-ahead window (blocks) to hide the hi AllGather


def _set_dims(n, e, split=None, tc=None):
    """Test hook: shrink the problem (keeps feature dims)."""
    global N, E, NPER, NB, NPAD, TC
    N, E = n, e
    NPER = N // NCORES
    NB = (NPER + BW - 1) // BW
    NPAD = NB * BW
    if tc is not None:
        TC = tc

def _halves():
    """Split the NB blocks into two AllGather halves (lo/hi tables)."""
    bl = (NB + 1) // 2
    return bl, NB - bl  # blocks in lo, hi


F16 = mybir.dt.float16
F32 = mybir.dt.float32
I16 = mybir.dt.int16
F8 = mybir.dt.float8e4
F8NP = mybir.dt.np(F8)  # ml_dtypes.float8_e4m3
FP8_ONE = 1.0

Alu = mybir.AluOpType
Act = mybir.ActivationFunctionType


def _wrap_idx(idx_stream):
    """int16 idx stream -> [128, len/16] wrapped layout (pos i -> [i%16, i//16]),
    replicated 8x across partition groups."""
    n = idx_stream.shape[0]
    assert n % 16 == 0
    a = idx_stream.reshape(n // 16, 16).T.astype(np.int16)
    return np.ascontiguousarray(np.tile(a, (8, 1)))


def _preprocess(inputs):
    """Host-side graph preprocessing. Returns (structure, per-core in_maps)."""
    x = np.asarray(inputs["x"], np.float32)
    edge_index = np.asarray(inputs["edge_index"], np.int64)
    src = np.concatenate([edge_index[0], np.arange(N, dtype=np.int64)])
    dst = np.concatenate([edge_index[1], np.arange(N, dtype=np.int64)])
    deg = np.bincount(dst, minlength=N).astype(np.float32)
    dinv = (1.0 / np.sqrt(deg)).astype(np.float32)

    order = np.argsort(dst, kind="stable")
    src_s = src[order]
    dst_s = dst[order]

    # self-edges (src==dst, incl. the synthetic self-loops) are folded into
    # the block epilogue as selfw[d] * h'_d instead of being gathered
    selfw_all = np.zeros(N, dtype=np.float32)
    self_m = src_s == dst_s
    np.add.at(selfw_all, dst_s[self_m], 1.0)
    src_s = src_s[~self_m]
    dst_s = dst_s[~self_m]

    # two-table AllGather layout: source node (c, r) lives in the lo table at
    # c*RLO + r (r < RLO) or the hi table at c*RHI + (r - RLO)
    BLO, BHI = _halves()
    RLO, RHI = BLO * 128, BHI * 128

    core_edge_start = np.searchsorted(dst_s, np.arange(0, N + 1, NPER))
    per_core = []
    nlo = np.zeros((NCORES, NB), dtype=np.int64)
    nhi = np.zeros((NCORES, NB), dtype=np.int64)
    for c in range(NCORES):
        s0, s1 = core_edge_start[c], core_edge_start[c + 1]
        sc, sr = src_s[s0:s1] // NPER, src_s[s0:s1] % NPER
        lo = sr < RLO
        cs = np.where(lo, sc * RLO + sr, sc * RHI + (sr - RLO))
        cd = dst_s[s0:s1] - c * NPER
        so = np.argsort(cs, kind="stable")  # src-sorted for HBM locality
        cs, cd, lo = cs[so], cd[so], lo[so]
        blk = cd // BW
        lists = []
        for b in range(NB):
            m = blk == b
            mlo = m & lo
            mhi = m & ~lo
            lists.append((cs[mlo], cd[mlo], cs[mhi], cd[mhi]))
            nlo[c, b] = int(mlo.sum())
            nhi[c, b] = int(mhi.sum())
        per_core.append(lists)

    Tlo = np.ceil(nlo.max(axis=0) / 128).astype(int)
    Thi = np.ceil(nhi.max(axis=0) / 128).astype(int)
    TLO, THI = int(Tlo.sum()), int(Thi.sum())
    lo_t0 = np.concatenate([[0], np.cumsum(Tlo)])[:-1]
    hi_t0 = np.concatenate([[0], np.cumsum(Thi)])[:-1]

    structure = dict(Tlo=Tlo.tolist(), Thi=Thi.tolist(), TLO=TLO, THI=THI,
                     lo_t0=lo_t0.tolist(), hi_t0=hi_t0.tolist())

    k = 1.0 / np.sqrt(1.0 + EPS)

    def fold(w, b, g, be):
        A = (np.asarray(g, np.float32) * k)
        Wp = (np.asarray(w, np.float32) * A[None, :]).astype(np.float16)
        B = (np.asarray(b, np.float32) * A + np.asarray(be, np.float32))
        return Wp, np.tile(B[None, :].astype(np.float32), (128, 1))

    w1p, b1rep = fold(inputs["w1"], inputs["b1"], inputs["g1"], inputs["be1"])
    w2p, b2rep = fold(inputs["w2"], inputs["b2"], inputs["g2"], inputs["be2"])
    w3p, b3rep = fold(inputs["w3"], inputs["b3"], inputs["g3"], inputs["be3"])

    lw1 = np.asarray(inputs["lw1"], np.float32).astype(np.float16)
    lw2 = np.asarray(inputs["lw2"], np.float32).astype(np.float16)
    lw3 = np.asarray(inputs["lw3"], np.float32).astype(np.float16)
    lw4 = np.asarray(inputs["lw4"], np.float32).astype(np.float16)

    shared = {
        "w1p": w1p, "w2p": w2p, "w3p": w3p,
        "b1rep": b1rep, "b2rep": b2rep, "b3rep": b3rep,
        "lw1a": np.ascontiguousarray(lw1[:, :128]),
        "lw1b": np.ascontiguousarray(lw1[:, 128:]),
        "lw2a": np.ascontiguousarray(lw2[:128, :]),
        "lw2b": np.ascontiguousarray(lw2[128:, :]),
        "lw3": lw3, "lw4": lw4,
        "lb1a": np.asarray(inputs["lb1"], np.float32)[:128, None].copy(),
        "lb1b": np.asarray(inputs["lb1"], np.float32)[128:, None].copy(),
        "lb2": np.asarray(inputs["lb2"], np.float32)[:, None].copy(),
        "lb3": np.asarray(inputs["lb3"], np.float32)[:, None].copy(),
        "lb4": np.asarray(inputs["lb4"], np.float32)[:, None].copy(),
        "ident": np.eye(128, dtype=np.float16),
        "iota": np.tile(np.arange(128, dtype=np.float16)[None, :], (128, 1)),
    }

    in_maps = []
    for c in range(NCORES):
        idx_lo = np.zeros(max(TLO, 1) * 128, dtype=np.int16)
        idx_hi = np.zeros(max(THI, 1) * 128, dtype=np.int16)
        # dst-local id streams: did[p, t] = local dst (0..127) of edge (t, p);
        # sentinel 300 for pad slots (never matches the 0..127 iota row)
        did_lo = np.full((128, max(TLO, 1)), 300.0, dtype=np.float16)
        did_hi = np.full((128, max(THI, 1)), 300.0, dtype=np.float16)
        for b in range(NB):
            cs_lo, cd_lo, cs_hi, cd_hi = per_core[c][b]
            o = lo_t0[b] * 128
            idx_lo[o:o + len(cs_lo)] = cs_lo.astype(np.int16)
            pos = o + np.arange(len(cd_lo))
            did_lo[pos % 128, pos // 128] = (cd_lo - b * BW).astype(np.float16)
            o = hi_t0[b] * 128
            idx_hi[o:o + len(cs_hi)] = cs_hi.astype(np.int16)
            pos = o + np.arange(len(cd_hi))
            did_hi[pos % 128, pos // 128] = (cd_hi - b * BW).astype(np.float16)

        dv = np.ones(NPAD, dtype=np.float32)
        dv[:NPER] = dinv[c * NPER:(c + 1) * NPER]
        dinv_loc = np.ascontiguousarray(dv.reshape(NB, 128).T)

        sw = np.zeros(NPAD, dtype=np.float32)
        sw[:NPER] = selfw_all[c * NPER:(c + 1) * NPER]
        selfw_loc = np.ascontiguousarray(sw.reshape(NB, 128).T)

        xT = np.zeros((IN, NPAD), dtype=np.float16)
        xT[:, :NPER] = x[c * NPER:(c + 1) * NPER].T.astype(np.float16)

        m = dict(shared)
        m.update({
            "xT": xT,
            "dinv": dinv_loc,
            "selfw": selfw_loc,
            "idxlo": _wrap_idx(idx_lo),
            "idxhi": _wrap_idx(idx_hi),
            "dlo": did_lo,
            "dhi": did_hi,
        })
        in_maps.append(m)

    return structure, in_maps


def _build(structure):
    """Build the SPMD Bass graph (shared by all 8 cores)."""
    Tlo, Thi = structure["Tlo"], structure["Thi"]
    TLO, THI = structure["TLO"], structure["THI"]
    lo_t0, hi_t0 = structure["lo_t0"], structure["hi_t0"]
    TLOp, THIp = max(TLO, 1), max(THI, 1)
    cores = list(range(NCORES))

    nc = bacc.Bacc("TRN2", target_bir_lowering=False, debug=False,
                   num_devices=NCORES, num_swdge_queues=4,
                   dynamic_dma_scratch_size=57344)

    P = {}
    def par(name, shape, dtype, out=False):
        P[name] = nc.declare_dram_parameter(name, shape, dtype, isOutput=out)
        return P[name]

    par("xT", [IN, NPAD], F16)
    par("w1p", [IN, HC], F16); par("w2p", [HC, HC], F16); par("w3p", [HC, HC], F16)
    par("b1rep", [128, HC], F32); par("b2rep", [128, HC], F32); par("b3rep", [128, HC], F32)
    par("dinv", [128, NB], F32)
    par("selfw", [128, NB], F32)
    par("ident", [128, 128], F16)
    par("iota", [128, 128], F16)
    par("idxlo", [128, 8 * TLOp], I16); par("idxhi", [128, 8 * THIp], I16)
    par("dlo", [128, TLOp], F16); par("dhi", [128, THIp], F16)
    par("lw1a", [HC, 128], F16); par("lw1b", [HC, FC1 - 128], F16)
    par("lw2a", [128, FC2], F16); par("lw2b", [FC1 - 128, FC2], F16)
    par("lw3", [FC2, FC3], F16); par("lw4", [FC3, OUT], F16)
    par("lb1a", [128, 1], F32); par("lb1b", [FC1 - 128, 1], F32)
    par("lb2", [FC2, 1], F32); par("lb3", [FC3, 1], F32); par("lb4", [OUT, 1], F32)
    par("out", [OUT, NPAD], F32, out=True)

    with tile.TileContext(nc) as tc:
        with (
            tc.tile_pool(name="const", bufs=1) as cpool,
            tc.tile_pool(name="dram", bufs=1, space="DRAM") as dpool,
            tc.tile_pool(name="hp", bufs=3) as hp_pool,
            tc.tile_pool(name="zt", bufs=2) as zt_pool,
            tc.tile_pool(name="xt", bufs=3) as xt_pool,
            tc.tile_pool(name="glo", bufs=4) as glo_pool,
            tc.tile_pool(name="ghi", bufs=4) as ghi_pool,
            tc.tile_pool(name="ind", bufs=3) as ind_pool,
            tc.tile_pool(name="tmp", bufs=4) as tmp_pool,
            tc.tile_pool(name="lop", bufs=34) as lop_pool,
            tc.tile_pool(name="mz", bufs=2) as mz_pool,
            tc.tile_pool(name="osb", bufs=2) as osb_pool,
            tc.tile_pool(name="psd", bufs=1, space="PSUM") as psd_pool,
            tc.tile_pool(name="psl", bufs=2, space="PSUM") as psl_pool,
            tc.tile_pool(name="psh", bufs=2, space="PSUM") as psh_pool,
            tc.tile_pool(name="ptr", bufs=1, space="PSUM") as ptr_pool,
        ):
            C = {}
            for name, p in P.items():
                if name in ("out", "xT"):  # xT is streamed per dense block
                    continue
                t = cpool.tile(list(p.shape), p.dtype, name=f"c_{name}")
                nc.sync.dma_start(t[:], p[:])
                C[name] = t

            BLO, BHI = _halves()
            RLO, RHI = BLO * 128, BHI * 128
            agin = [[dpool.tile([RLO, 128], F16, name=f"aginlo{l}"),
                     dpool.tile([RHI, 128], F16, name=f"aginhi{l}")]
                    for l in range(3)]
            hlo = [dpool.tile([NCORES * RLO, 128], F16, addr_space="Shared",
                              name=f"hlo{l}") for l in range(3)]
            hhi = [dpool.tile([NCORES * RHI, 128], F16, addr_space="Shared",
                              name=f"hhi{l}") for l in range(3)]

            w_sb = [C["w1p"], C["w2p"], C["w3p"]]
            b_sb = [C["b1rep"], C["b2rep"], C["b3rep"]]
            dinv_sb = C["dinv"]

            def big_alloc(name):
                """[128, NPAD] fp16 big tile with pad cols (96:128) zeroed."""
                t = hp_pool.tile([128, NPAD], F16, tag="hp", name=name)
                nc.vector.memset(
                    t[:].rearrange("p (t f) -> p t f", f=128)[:, :, HC:128], 0.0)
                return t

            def emit_half_ag(l, hp, b):
                """After block b (a half boundary), ship that half + AllGather."""
                if b == BLO - 1:
                    k, b0, nb, tbl = 0, 0, BLO, hlo[l]
                elif b == NB - 1:
                    k, b0, nb, tbl = 1, BLO, BHI, hhi[l]
                else:
                    return
                ag = agin[l][k]
                nc.sync.dma_start(
                    ag[:].rearrange("(t p) f -> p t f", p=128),
                    hp[:, b0 * 128:(b0 + nb) * 128].rearrange(
                        "p (t f) -> p t f", f=128))
                nc.gpsimd.collective_compute(
                    "AllGather", Alu.bypass, replica_groups=[cores],
                    ins=[ag[:]], outs=[tbl[:]])

            def dense_block(l, src_ap, hp, t, scale_dinv):
                ps = psd_pool.tile([128, HC], F32, tag="psd",
                                   name=f"psd{l}_{t}")
                nc.tensor.matmul(ps[:], src_ap(t), w_sb[l][:],
                                 start=True, stop=True)
                if scale_dinv:
                    nc.vector.tensor_scalar(
                        hp[:, t * 128:t * 128 + HC], ps[:],
                        dinv_sb[:, t:t + 1], None, Alu.mult)
                else:
                    nc.vector.tensor_copy(hp[:, t * 128:t * 128 + HC], ps[:])
                emit_half_ag(l, hp, t)

            def x_src(t):
                xt = xt_pool.tile([IN, 128], F16, tag="xt", name=f"xt{t}")
                nc.sync.dma_start(xt[:], P["xT"][:, t * 128:(t + 1) * 128])
                return xt[:]

            def make_dense_consumer(l1, fin):
                hp = big_alloc(f"hpd{l1}")
                def on_block(b, zt):
                    dense_block(l1, lambda t: zt[0:fin, t * 128:(t + 1) * 128],
                                hp, b, scale_dinv=False)
                return hp, on_block

            # MLP head (feature-major), consumes y3t incrementally
            out_mlp_state = {}
            def mlp_chunk(r0, cw, y3t):
                rs = slice(r0, r0 + cw)
                p1a = psd_pool.tile([128, 512], F32, tag="mps", bufs=2)
                nc.tensor.matmul(p1a[:, 0:cw], C["lw1a"][:], y3t[0:HC, rs],
                                 start=True, stop=True)
                z1a = mz_pool.tile([128, 512], F16, tag="z1a")
                nc.scalar.activation(z1a[:, 0:cw], p1a[:, 0:cw], Act.Relu,
                                     bias=C["lb1a"][:])
                p1b = psd_pool.tile([128, 512], F32, tag="mps", bufs=2)
                nc.tensor.matmul(p1b[:, 0:cw], C["lw1b"][:], y3t[0:HC, rs],
                                 start=True, stop=True)
                z1b = mz_pool.tile([128, 512], F16, tag="z1b")
                nc.scalar.activation(z1b[:, 0:cw], p1b[:, 0:cw], Act.Relu,
                                     bias=C["lb1b"][:])
                p2 = psd_pool.tile([128, 512], F32, tag="mps", bufs=2)
                nc.tensor.matmul(p2[:, 0:cw], C["lw2a"][:], z1a[:, 0:cw],
                                 start=True, stop=False)
                nc.tensor.matmul(p2[:, 0:cw], C["lw2b"][:], z1b[:, 0:cw],
                                 start=False, stop=True)
                z2m = mz_pool.tile([128, 512], F16, tag="z2m")
                nc.scalar.activation(z2m[:, 0:cw], p2[:, 0:cw], Act.Relu,
                                     bias=C["lb2"][:])
                p3 = psd_pool.tile([FC3, 512], F32, tag="mps", bufs=2)
                nc.tensor.matmul(p3[:, 0:cw], C["lw3"][:], z2m[:, 0:cw],
                                 start=True, stop=True)
                z3m = mz_pool.tile([FC3, 512], F16, tag="z3m")
                nc.scalar.activation(z3m[:, 0:cw], p3[:, 0:cw], Act.Relu,
                                     bias=C["lb3"][:])
                p4 = psd_pool.tile([OUT, 512], F32, tag="mps", bufs=2)
                nc.tensor.matmul(p4[:, 0:cw], C["lw4"][:], z3m[:, 0:cw],
                                 start=True, stop=True)
                ob = osb_pool.tile([OUT, 512], F32, tag="ob")
                nc.vector.tensor_scalar(ob[:, 0:cw], p4[:, 0:cw], C["lb4"][:],
                                        None, Alu.add)
                nc.sync.dma_start(P["out"][:, rs], ob[:, 0:cw])

            def mlp_consumer(b, y3t):
                # emit an MLP chunk whenever 4 more blocks (512 cols) are ready
                r0 = (b // 4) * 512
                if b == NB - 1:
                    while r0 < NPAD:
                        mlp_chunk(r0, min(512, NPAD - r0), y3t)
                        r0 += 512
                        out_mlp_state["r0"] = r0
                elif b % 4 == 3:
                    mlp_chunk(r0, 512, y3t)
                    out_mlp_state["r0"] = r0 + 512

            qctr = [0]

            def gather_stage(l, premult, hp_own, on_block):
                """Aggregate edges against tables l; returns (out_big, zt)."""
                out_big = big_alloc(f"agg{l}")
                zt = zt_pool.tile([128, NPAD], F16, tag="zt", name=f"zt{l}")
                streams = {
                    "lo": dict(T=TLO, t0=lo_t0, nt=Tlo, did=C["dlo"],
                               idx=C["idxlo"], src=hlo[l][:],
                               pool=glo_pool, cache={}),
                    "hi": dict(T=THI, t0=hi_t0, nt=Thi, did=C["dhi"],
                               idx=C["idxhi"], src=hhi[l][:],
                               pool=ghi_pool, cache={}),
                }

                def fetch(s, sname, k):
                    if k in s["cache"]:
                        return s["cache"][k]
                    t0 = k * TC
                    ct = min(TC, s["T"] - t0)
                    nidx = ct * 128
                    g = s["pool"].tile([128, TC * 128], F16, tag=f"g{sname}",
                                       name=f"g{sname}_{l}_{k}")
                    nc.gpsimd.dma_gather(
                        g[:, 0:nidx].rearrange("p (t e) -> p t e", e=128),
                        s["src"],
                        s["idx"][:, t0 * 8:(t0 + ct) * 8],
                        nidx, nidx, 128,
                        queue_num=qctr[0] % 4)
                    qctr[0] += 1
                    ind = ind_pool.tile([128, TC * 128], F8, tag=f"i{sname}",
                                        name=f"i{sname}_{l}_{k}")
                    # on-chip one-hot: ind[p, t, d] = (did[p, t0+t] == d)
                    nc.vector.tensor_tensor(
                        ind[:, 0:nidx].rearrange("p (t e) -> p t e", e=128),
                        C["iota"][:].rearrange("p (t e) -> p t e", t=1)
                            .to_broadcast([128, ct, 128]),
                        s["did"][:, t0:t0 + ct].to_broadcast([128, ct, 128]),
                        Alu.is_equal)
                    s["cache"][k] = (g, ind)
                    return s["cache"][k]

                def emit_chain(b, sname, pool, ptag):
                    """Accumulate stream `sname`'s tiles of block b in PSUM."""
                    s = streams[sname]
                    nt = s["nt"][b]
                    ps = pool.tile([128, HC], F32, tag=ptag,
                                   name=f"{ptag}{l}_{b}")
                    for i in range(nt):
                        t = s["t0"][b] + i
                        k = t // TC
                        g, ind = fetch(s, sname, k)
                        jj = t - k * TC
                        nc.tensor.matmul(
                            ps[:], ind[:, jj * 128:(jj + 1) * 128],
                            g[:, jj * 128:jj * 128 + HC],
                            start=(i == 0), stop=(i == nt - 1))
                    return ps

                pslo = {}

                def emit_lo(b):
                    # lo chain accumulates in PSUM, then spills to SBUF so the
                    # bank frees immediately (keeps the lo-ahead window deep)
                    if Tlo[b] == 0:
                        return
                    ps = emit_chain(b, "lo", psl_pool, "pl")
                    sp = lop_pool.tile([128, HC], F16, tag="lop",
                                       name=f"lop{l}_{b}")
                    nc.vector.tensor_copy(sp[:], ps[:])
                    pslo[b] = sp

                def emit_full(b):
                    """Baseline path: one PSUM chain over lo+hi tiles."""
                    ntiles = Tlo[b] + Thi[b]
                    ps = None
                    if ntiles > 0:
                        ps = psh_pool.tile([128, HC], F32, tag="ph",
                                           name=f"ph{l}_{b}")
                        i = 0
                        for sname in ("lo", "hi"):
                            s = streams[sname]
                            for j in range(s["nt"][b]):
                                t = s["t0"][b] + j
                                k = t // TC
                                g, ind = fetch(s, sname, k)
                                jj = t - k * TC
                                nc.tensor.matmul(
                                    ps[:], ind[:, jj * 128:(jj + 1) * 128],
                                    g[:, jj * 128:jj * 128 + HC],
                                    start=(i == 0), stop=(i == ntiles - 1))
                                i += 1
                    epilogue(b, ps, None)

                def emit_hi_epi(b):
                    ps = None
                    if Thi[b] > 0:
                        ps = emit_chain(b, "hi", psh_pool, "ph")
                    epilogue(b, ps, pslo.pop(b, None))

                def epilogue(b, ps, sp):
                    tmp = tmp_pool.tile([128, HC], F32, tag="tmp")
                    tmp0 = tmp_pool.tile([128, HC], F32, tag="tmp0")
                    hps = hp_own[:, b * 128:b * 128 + HC]
                    first = ps[:] if ps is not None else (
                        sp[:] if sp is not None else None)
                    if first is not None:
                        # S + selfw*h_own  (self-edges folded out of the gather)
                        nc.vector.scalar_tensor_tensor(
                            tmp0[:], hps, C["selfw"][:, b:b + 1], first,
                            Alu.mult, Alu.add)
                    else:
                        nc.vector.tensor_scalar(
                            tmp0[:], hps, C["selfw"][:, b:b + 1], None, Alu.mult)
                    if ps is not None and sp is not None:
                        tmpa = tmp_pool.tile([128, HC], F32, tag="tmpa")
                        nc.vector.tensor_tensor(tmpa[:], tmp0[:], sp[:], Alu.add)
                        tmp0 = tmpa
                    nc.vector.scalar_tensor_tensor(
                        tmp[:], tmp0[:], dinv_sb[:, b:b + 1], b_sb[l][:],
                        Alu.mult, Alu.add)
                    if premult:
                        nc.scalar.activation(
                            out_big[:, b * 128:b * 128 + HC], tmp[:], Act.Relu,
                            scale=dinv_sb[:, b:b + 1])
                    else:
                        nc.scalar.activation(
                            out_big[:, b * 128:b * 128 + HC], tmp[:], Act.Relu)
                    # transposed copy for the next dense / MLP (feature-major)
                    ptr = ptr_pool.tile([128, 128], F16, tag="ptr", space="PSUM")
                    nc.tensor.transpose(
                        ptr[:], out_big[:, b * 128:(b + 1) * 128], C["ident"][:])
                    nc.vector.tensor_copy(zt[:, b * 128:(b + 1) * 128], ptr[:])
                    on_block(b, zt)

                # lo-ahead window: the first W blocks' lo-stream gathers are
                # emitted before any hi-dependent work so gpsimd stays busy
                # while the hi-half AllGather is still in flight; the rest of
                # the stage uses the baseline single-chain path
                for b in range(min(W, NB)):
                    emit_lo(b)
                for b in range(NB):
                    if b < W:
                        emit_hi_epi(b)
                    else:
                        emit_full(b)
                return out_big, zt

            # layer 1 dense (x streamed per block from DRAM)
            hp1 = big_alloc("hpd0")
            for t in range(NB):
                dense_block(0, x_src, hp1, t, scale_dinv=True)

            hp2, ob1 = make_dense_consumer(1, HC)
            z2, zt2 = gather_stage(0, True, hp1, ob1)
            hp3, ob2 = make_dense_consumer(2, HC)
            z3, zt3 = gather_stage(1, True, hp2, ob2)
            y3, y3t = gather_stage(2, False, hp3, mlp_consumer)

    nc.compile()
    return nc


_CACHE = {}


def kernel(**inputs):
    structure, in_maps = _preprocess(inputs)
    key = (tuple(structure["Tlo"]), tuple(structure["Thi"]))
    if key not in _CACHE:
        _CACHE[key] = _build(structure)
    nc = _CACHE[key]
    res = run_bass_kernel_spmd(nc, in_maps, core_ids=list(range(NCORES)))
    out = np.empty((N, OUT), np.float32)
    for c in range(NCORES):
        out[c * NPER:(c + 1) * NPER] = res.results[c]["out"].T[:NPER]
    return out



# revision 30
# speedup vs baseline: 1.1444x; 1.1444x over previous
"""GCN message-passing kernel for 8 TRN2 NeuronCores.

Strategy (graph/data parallel, dst-sharded):
  - Nodes sharded 6250/core. Per layer: each core computes its shard of the
    gather table H' (dense matmul + epilogue), an AllGather builds the full
    50000x128 fp16 table in HBM.
  - Edges (incl. self-loops) sorted by dst, grouped into 128-wide dst blocks.
    Per 128-edge tile: dma_gather fetches H'[src] rows (256B fp16), DVE
    is_equal builds the one-hot [edge, dst_local] tile, PE accumulates
    Onehot^T @ G into the block's PSUM (segment sum).
  - GCN norm dinv[src]*dinv[dst] is separable: src side folded into the
    table rows, dst side into the block epilogue. BN folded into W and B.
  - Epilogues: dense1: tbl = dinv*(x@W1A); gather1/2: z = dinv*relu(S*dinv+B)
    (the outer dinv pre-applies the next table's src factor); dense2/3 are
    then plain copies; gather3: y3 = relu(S*dinv+B) feeds the MLP head.
  - 4-layer MLP runs feature-major per core; output [8, 6272] f32 is
    transposed/trimmed on the host.
"""

import sys

sys.path.insert(0, "/opt/trn_rl_repo")

import numpy as np

import concourse.bacc as bacc
import concourse.mybir as mybir
import concourse.tile as tile
from concourse.bass_utils import run_bass_kernel_spmd

# Problem constants
N, E, IN, HC = 50000, 800000, 64, 96
FC1, FC2, FC3, OUT = 256, 128, 64, 8
EPS = 1e-5
NCORES = 8
NPER = N // NCORES          # 6250 nodes per core
BW = 128                    # dst-block width
NB = (NPER + BW - 1) // BW  # 49 blocks
NPAD = NB * BW              # 6272
TC = 8                      # tiles per gather chunk (1024 idxs; >=1920 hangs HW)
W = 15                      # lo-ahead window (blocks) to hide the hi AllGather


def _set_dims(n, e, split=None, tc=None):
    """Test hook: shrink the problem (keeps feature dims)."""
    global N, E, NPER, NB, NPAD, TC
    N, E = n, e
    NPER = N // NCORES
    NB = (NPER + BW - 1) // BW
    NPAD = NB * BW
    if tc is not None:
        TC = tc

def _halves():
    """Split the NB blocks into two AllGather halves (lo/hi tables)."""
    bl = (NB + 1) // 2
    return bl, NB - bl  # blocks in lo, hi


F16 = mybir.dt.float16
F32 = mybir.dt.float32
I16 = mybir.dt.int16
F8 = mybir.dt.float8e4
F8NP = mybir.dt.np(F8)  # ml_dtypes.float8_e4m3
FP8_ONE = 1.0

Alu = mybir.AluOpType
Act = mybir.ActivationFunctionType


def _wrap_idx(idx_stream):
    """int16 idx stream -> [128, len/16] wrapped layout (pos i -> [i%16, i//16]),
    replicated 8x across partition groups."""
    n = idx_stream.shape[0]
    assert n % 16 == 0
    a = idx_stream.reshape(n // 16, 16).T.astype(np.int16)
    return np.ascontiguousarray(np.tile(a, (8, 1)))


def _preprocess(inputs):
    """Host-side graph preprocessing. Returns (structure, per-core in_maps)."""
    x = np.asarray(inputs["x"], np.float32)
    edge_index = np.asarray(inputs["edge_index"], np.int64)
    src = np.concatenate([edge_index[0], np.arange(N, dtype=np.int64)])
    dst = np.concatenate([edge_index[1], np.arange(N, dtype=np.int64)])
    deg = np.bincount(dst, minlength=N).astype(np.float32)
    dinv = (1.0 / np.sqrt(deg)).astype(np.float32)

    order = np.argsort(dst, kind="stable")
    src_s = src[order]
    dst_s = dst[order]

    # self-edges (src==dst, incl. the synthetic self-loops) are folded into
    # the block epilogue as selfw[d] * h'_d instead of being gathered
    selfw_all = np.zeros(N, dtype=np.float32)
    self_m = src_s == dst_s
    np.add.at(selfw_all, dst_s[self_m], 1.0)
    src_s = src_s[~self_m]
    dst_s = dst_s[~self_m]

    # two-table AllGather layout: source node (c, r) lives in the lo table at
    # c*RLO + r (r < RLO) or the hi table at c*RHI + (r - RLO)
    BLO, BHI = _halves()
    RLO, RHI = BLO * 128, BHI * 128

    core_edge_start = np.searchsorted(dst_s, np.arange(0, N + 1, NPER))
    per_core = []
    nlo = np.zeros((NCORES, NB), dtype=np.int64)
    nhi = np.zeros((NCORES, NB), dtype=np.int64)
    for c in range(NCORES):
        s0, s1 = core_edge_start[c], core_edge_start[c + 1]
        sc, sr = src_s[s0:s1] // NPER, src_s[s0:s1] % NPER
        lo = sr < RLO
        cs = np.where(lo, sc * RLO + sr, sc * RHI + (sr - RLO))
        cd = dst_s[s0:s1] - c * NPER
        so = np.argsort(cs, kind="stable")  # src-sorted for HBM locality
        cs, cd, lo = cs[so], cd[so], lo[so]
        blk = cd // BW
        lists = []
        for b in range(NB):
            m = blk == b
            mlo = m & lo
            mhi = m & ~lo
            lists.append((cs[mlo], cd[mlo], cs[mhi], cd[mhi]))
            nlo[c, b] = int(mlo.sum())
            nhi[c, b] = int(mhi.sum())
        per_core.append(lists)

    Tlo = np.ceil(nlo.max(axis=0) / 128).astype(int)
    Thi = np.ceil(nhi.max(axis=0) / 128).astype(int)
    TLO, THI = int(Tlo.sum()), int(Thi.sum())
    lo_t0 = np.concatenate([[0], np.cumsum(Tlo)])[:-1]
    hi_t0 = np.concatenate([[0], np.cumsum(Thi)])[:-1]

    structure = dict(Tlo=Tlo.tolist(), Thi=Thi.tolist(), TLO=TLO, THI=THI,
                     lo_t0=lo_t0.tolist(), hi_t0=hi_t0.tolist())

    k = 1.0 / np.sqrt(1.0 + EPS)

    def fold(w, b, g, be):
        A = (np.asarray(g, np.float32) * k)
        Wp = (np.asarray(w, np.float32) * A[None, :]).astype(np.float16)
        B = (np.asarray(b, np.float32) * A + np.asarray(be, np.float32))
        return Wp, np.tile(B[None, :].astype(np.float32), (128, 1))

    w1p, b1rep = fold(inputs["w1"], inputs["b1"], inputs["g1"], inputs["be1"])
    w2p, b2rep = fold(inputs["w2"], inputs["b2"], inputs["g2"], inputs["be2"])
    w3p, b3rep = fold(inputs["w3"], inputs["b3"], inputs["g3"], inputs["be3"])

    lw1 = np.asarray(inputs["lw1"], np.float32).astype(np.float16)
    lw2 = np.asarray(inputs["lw2"], np.float32).astype(np.float16)
    lw3 = np.asarray(inputs["lw3"], np.float32).astype(np.float16)
    lw4 = np.asarray(inputs["lw4"], np.float32).astype(np.float16)

    shared = {
        "w1p": w1p, "w2p": w2p, "w3p": w3p,
        "b1rep": b1rep, "b2rep": b2rep, "b3rep": b3rep,
        "lw1a": np.ascontiguousarray(lw1[:, :128]),
        "lw1b": np.ascontiguousarray(lw1[:, 128:]),
        "lw2a": np.ascontiguousarray(lw2[:128, :]),
        "lw2b": np.ascontiguousarray(lw2[128:, :]),
        "lw3": lw3, "lw4": lw4,
        "lb1a": np.asarray(inputs["lb1"], np.float32)[:128, None].copy(),
        "lb1b": np.asarray(inputs["lb1"], np.float32)[128:, None].copy(),
        "lb2": np.asarray(inputs["lb2"], np.float32)[:, None].copy(),
        "lb3": np.asarray(inputs["lb3"], np.float32)[:, None].copy(),
        "lb4": np.asarray(inputs["lb4"], np.float32)[:, None].copy(),
        "ident": np.eye(128, dtype=np.float16),
        "iota": np.tile(np.arange(128, dtype=np.float16)[None, :], (128, 1)),
    }

    in_maps = []
    for c in range(NCORES):
        idx_lo = np.zeros(max(TLO, 1) * 128, dtype=np.int16)
        idx_hi = np.zeros(max(THI, 1) * 128, dtype=np.int16)
        # dst-local id streams: did[p, t] = local dst (0..127) of edge (t, p);
        # sentinel 300 for pad slots (never matches the 0..127 iota row)
        did_lo = np.full((128, max(TLO, 1)), 300.0, dtype=np.float16)
        did_hi = np.full((128, max(THI, 1)), 300.0, dtype=np.float16)
        for b in range(NB):
            cs_lo, cd_lo, cs_hi, cd_hi = per_core[c][b]
            o = lo_t0[b] * 128
            idx_lo[o:o + len(cs_lo)] = cs_lo.astype(np.int16)
            pos = o + np.arange(len(cd_lo))
            did_lo[pos % 128, pos // 128] = (cd_lo - b * BW).astype(np.float16)
            o = hi_t0[b] * 128
            idx_hi[o:o + len(cs_hi)] = cs_hi.astype(np.int16)
            pos = o + np.arange(len(cd_hi))
            did_hi[pos % 128, pos // 128] = (cd_hi - b * BW).astype(np.float16)

        dv = np.ones(NPAD, dtype=np.float32)
        dv[:NPER] = dinv[c * NPER:(c + 1) * NPER]
        dinv_loc = np.ascontiguousarray(dv.reshape(NB, 128).T)

        sw = np.zeros(NPAD, dtype=np.float32)
        sw[:NPER] = selfw_all[c * NPER:(c + 1) * NPER]
        selfw_loc = np.ascontiguousarray(sw.reshape(NB, 128).T)

        xT = np.zeros((IN, NPAD), dtype=np.float16)
        xT[:, :NPER] = x[c * NPER:(c + 1) * NPER].T.astype(np.float16)

        m = dict(shared)
        m.update({
            "xT": xT,
            "dinv": dinv_loc,
            "selfw": selfw_loc,
            "idxlo": _wrap_idx(idx_lo),
            "idxhi": _wrap_idx(idx_hi),
            "dlo": did_lo,
            "dhi": did_hi,
        })
        in_maps.append(m)

    return structure, in_maps


def _build(structure):
    """Build the SPMD Bass graph (shared by all 8 cores)."""
    Tlo, Thi = structure["Tlo"], structure["Thi"]
    TLO, THI = structure["TLO"], structure["THI"]
    lo_t0, hi_t0 = structure["lo_t0"], structure["hi_t0"]
    TLOp, THIp = max(TLO, 1), max(THI, 1)
    cores = list(range(NCORES))

    nc = bacc.Bacc("TRN2", target_bir_lowering=False, debug=False,
                   num_devices=NCORES, num_swdge_queues=4,
                   dynamic_dma_scratch_size=57344)

    P = {}
    def par(name, shape, dtype, out=False):
        P[name] = nc.declare_dram_parameter(name, shape, dtype, isOutput=out)
        return P[name]

    par("xT", [IN, NPAD], F16)
    par("w1p", [IN, HC], F16); par("w2p", [HC, HC], F16); par("w3p", [HC, HC], F16)
    par("b1rep", [128, HC], F32); par("b2rep", [128, HC], F32); par("b3rep", [128, HC], F32)
    par("dinv", [128, NB], F32)
    par("selfw", [128, NB], F32)
    par("ident", [128, 128], F16)
    par("iota", [128, 128], F16)
    par("idxlo", [128, 8 * TLOp], I16); par("idxhi", [128, 8 * THIp], I16)
    par("dlo", [128, TLOp], F16); par("dhi", [128, THIp], F16)
    par("lw1a", [HC, 128], F16); par("lw1b", [HC, FC1 - 128], F16)
    par("lw2a", [128, FC2], F16); par("lw2b", [FC1 - 128, FC2], F16)
    par("lw3", [FC2, FC3], F16); par("lw4", [FC3, OUT], F16)
    par("lb1a", [128, 1], F32); par("lb1b", [FC1 - 128, 1], F32)
    par("lb2", [FC2, 1], F32); par("lb3", [FC3, 1], F32); par("lb4", [OUT, 1], F32)
    par("out", [OUT, NPAD], F32, out=True)

    with tile.TileContext(nc) as tc:
        with (
            tc.tile_pool(name="const", bufs=1) as cpool,
            tc.tile_pool(name="dram", bufs=1, space="DRAM") as dpool,
            tc.tile_pool(name="hp", bufs=3) as hp_pool,
            tc.tile_pool(name="zt", bufs=2) as zt_pool,
            tc.tile_pool(name="xt", bufs=3) as xt_pool,
            tc.tile_pool(name="glo", bufs=4) as glo_pool,
            tc.tile_pool(name="ghi", bufs=4) as ghi_pool,
            tc.tile_pool(name="ind", bufs=3) as ind_pool,
            tc.tile_pool(name="tmp", bufs=4) as tmp_pool,
            tc.tile_pool(name="lop", bufs=18) as lop_pool,
            tc.tile_pool(name="mz", bufs=2) as mz_pool,
            tc.tile_pool(name="osb", bufs=2) as osb_pool,
            tc.tile_pool(name="psd", bufs=1, space="PSUM") as psd_pool,
            tc.tile_pool(name="psl", bufs=2, space="PSUM") as psl_pool,
            tc.tile_pool(name="psh", bufs=2, space="PSUM") as psh_pool,
            tc.tile_pool(name="ptr", bufs=1, space="PSUM") as ptr_pool,
        ):
            C = {}
            for name, p in P.items():
                if name in ("out", "xT"):  # xT is streamed per dense block
                    continue
                t = cpool.tile(list(p.shape), p.dtype, name=f"c_{name}")
                nc.sync.dma_start(t[:], p[:])
                C[name] = t

            BLO, BHI = _halves()
            RLO, RHI = BLO * 128, BHI * 128
            agin = [[dpool.tile([RLO, 128], F16, name=f"aginlo{l}"),
                     dpool.tile([RHI, 128], F16, name=f"aginhi{l}")]
                    for l in range(3)]
            hlo = [dpool.tile([NCORES * RLO, 128], F16, addr_space="Shared",
                              name=f"hlo{l}") for l in range(3)]
            hhi = [dpool.tile([NCORES * RHI, 128], F16, addr_space="Shared",
                              name=f"hhi{l}") for l in range(3)]

            w_sb = [C["w1p"], C["w2p"], C["w3p"]]
            b_sb = [C["b1rep"], C["b2rep"], C["b3rep"]]
            dinv_sb = C["dinv"]

            def big_alloc(name):
                """[128, NPAD] fp16 big tile with pad cols (96:128) zeroed."""
                t = hp_pool.tile([128, NPAD], F16, tag="hp", name=name)
                nc.vector.memset(
                    t[:].rearrange("p (t f) -> p t f", f=128)[:, :, HC:128], 0.0)
                return t

            def emit_half_ag(l, hp, b):
                """After block b (a half boundary), ship that half + AllGather."""
                if b == BLO - 1:
                    k, b0, nb, tbl = 0, 0, BLO, hlo[l]
                elif b == NB - 1:
                    k, b0, nb, tbl = 1, BLO, BHI, hhi[l]
                else:
                    return
                ag = agin[l][k]
                nc.sync.dma_start(
                    ag[:].rearrange("(t p) f -> p t f", p=128),
                    hp[:, b0 * 128:(b0 + nb) * 128].rearrange(
                        "p (t f) -> p t f", f=128))
                nc.gpsimd.collective_compute(
                    "AllGather", Alu.bypass, replica_groups=[cores],
                    ins=[ag[:]], outs=[tbl[:]])

            def dense_block(l, src_ap, hp, t, scale_dinv):
                ps = psd_pool.tile([128, HC], F32, tag="psd",
                                   name=f"psd{l}_{t}")
                nc.tensor.matmul(ps[:], src_ap(t), w_sb[l][:],
                                 start=True, stop=True)
                if scale_dinv:
                    nc.vector.tensor_scalar(
                        hp[:, t * 128:t * 128 + HC], ps[:],
                        dinv_sb[:, t:t + 1], None, Alu.mult)
                else:
                    nc.vector.tensor_copy(hp[:, t * 128:t * 128 + HC], ps[:])
                emit_half_ag(l, hp, t)

            def x_src(t):
                xt = xt_pool.tile([IN, 128], F16, tag="xt", name=f"xt{t}")
                nc.sync.dma_start(xt[:], P["xT"][:, t * 128:(t + 1) * 128])
                return xt[:]

            def make_dense_consumer(l1, fin):
                hp = big_alloc(f"hpd{l1}")
                def on_block(b, zt):
                    dense_block(l1, lambda t: zt[0:fin, t * 128:(t + 1) * 128],
                                hp, b, scale_dinv=False)
                return hp, on_block

            # MLP head (feature-major), consumes y3t incrementally
            out_mlp_state = {}
            def mlp_chunk(r0, cw, y3t):
                rs = slice(r0, r0 + cw)
                p1a = psd_pool.tile([128, 512], F32, tag="mps", bufs=2)
                nc.tensor.matmul(p1a[:, 0:cw], C["lw1a"][:], y3t[0:HC, rs],
                                 start=True, stop=True)
                z1a = mz_pool.tile([128, 512], F16, tag="z1a")
                nc.scalar.activation(z1a[:, 0:cw], p1a[:, 0:cw], Act.Relu,
                                     bias=C["lb1a"][:])
                p1b = psd_pool.tile([128, 512], F32, tag="mps", bufs=2)
                nc.tensor.matmul(p1b[:, 0:cw], C["lw1b"][:], y3t[0:HC, rs],
                                 start=True, stop=True)
                z1b = mz_pool.tile([128, 512], F16, tag="z1b")
                nc.scalar.activation(z1b[:, 0:cw], p1b[:, 0:cw], Act.Relu,
                                     bias=C["lb1b"][:])
                p2 = psd_pool.tile([128, 512], F32, tag="mps", bufs=2)
                nc.tensor.matmul(p2[:, 0:cw], C["lw2a"][:], z1a[:, 0:cw],
                                 start=True, stop=False)
                nc.tensor.matmul(p2[:, 0:cw], C["lw2b"][:], z1b[:, 0:cw],
                                 start=False, stop=True)
                z2m = mz_pool.tile([128, 512], F16, tag="z2m")
                nc.scalar.activation(z2m[:, 0:cw], p2[:, 0:cw], Act.Relu,
                                     bias=C["lb2"][:])
                p3 = psd_pool.tile([FC3, 512], F32, tag="mps", bufs=2)
                nc.tensor.matmul(p3[:, 0:cw], C["lw3"][:], z2m[:, 0:cw],
                                 start=True, stop=True)
                z3m = mz_pool.tile([FC3, 512], F16, tag="z3m")
                nc.scalar.activation(z3m[:, 0:cw], p3[:, 0:cw], Act.Relu,
                                     bias=C["lb3"][:])
                p4 = psd_pool.tile([OUT, 512], F32, tag="mps", bufs=2)
                nc.tensor.matmul(p4[:, 0:cw], C["lw4"][:], z3m[:, 0:cw],
                                 start=True, stop=True)
                ob = osb_pool.tile([OUT, 512], F32, tag="ob")
                nc.vector.tensor_scalar(ob[:, 0:cw], p4[:, 0:cw], C["lb4"][:],
                                        None, Alu.add)
                nc.sync.dma_start(P["out"][:, rs], ob[:, 0:cw])

            def mlp_consumer(b, y3t):
                # emit an MLP chunk whenever 4 more blocks (512 cols) are ready
                r0 = (b // 4) * 512
                if b == NB - 1:
                    while r0 < NPAD:
                        mlp_chunk(r0, min(512, NPAD - r0), y3t)
                        r0 += 512
                        out_mlp_state["r0"] = r0
                elif b % 4 == 3:
                    mlp_chunk(r0, 512, y3t)
                    out_mlp_state["r0"] = r0 + 512

            qctr = [0]

            def gather_stage(l, premult, hp_own, on_block):
                """Aggregate edges against tables l; returns (out_big, zt)."""
                out_big = big_alloc(f"agg{l}")
                zt = zt_pool.tile([128, NPAD], F16, tag="zt", name=f"zt{l}")
                streams = {
                    "lo": dict(T=TLO, t0=lo_t0, nt=Tlo, did=C["dlo"],
                               idx=C["idxlo"], src=hlo[l][:],
                               pool=glo_pool, cache={}),
                    "hi": dict(T=THI, t0=hi_t0, nt=Thi, did=C["dhi"],
                               idx=C["idxhi"], src=hhi[l][:],
                               pool=ghi_pool, cache={}),
                }

                def fetch(s, sname, k):
                    if k in s["cache"]:
                        return s["cache"][k]
                    t0 = k * TC
                    ct = min(TC, s["T"] - t0)
                    nidx = ct * 128
                    g = s["pool"].tile([128, TC * 128], F16, tag=f"g{sname}",
                                       name=f"g{sname}_{l}_{k}")
                    nc.gpsimd.dma_gather(
                        g[:, 0:nidx].rearrange("p (t e) -> p t e", e=128),
                        s["src"],
                        s["idx"][:, t0 * 8:(t0 + ct) * 8],
                        nidx, nidx, 128,
                        queue_num=qctr[0] % 4)
                    qctr[0] += 1
                    ind = ind_pool.tile([128, TC * 128], F8, tag=f"i{sname}",
                                        name=f"i{sname}_{l}_{k}")
                    # on-chip one-hot: ind[p, t, d] = (did[p, t0+t] == d)
                    nc.vector.tensor_tensor(
                        ind[:, 0:nidx].rearrange("p (t e) -> p t e", e=128),
                        C["iota"][:].rearrange("p (t e) -> p t e", t=1)
                            .to_broadcast([128, ct, 128]),
                        s["did"][:, t0:t0 + ct].to_broadcast([128, ct, 128]),
                        Alu.is_equal)
                    s["cache"][k] = (g, ind)
                    return s["cache"][k]

                def emit_chain(b, sname, pool, ptag):
                    """Accumulate stream `sname`'s tiles of block b in PSUM."""
                    s = streams[sname]
                    nt = s["nt"][b]
                    ps = pool.tile([128, HC], F32, tag=ptag,
                                   name=f"{ptag}{l}_{b}")
                    for i in range(nt):
                        t = s["t0"][b] + i
                        k = t // TC
                        g, ind = fetch(s, sname, k)
                        jj = t - k * TC
                        nc.tensor.matmul(
                            ps[:], ind[:, jj * 128:(jj + 1) * 128],
                            g[:, jj * 128:jj * 128 + HC],
                            start=(i == 0), stop=(i == nt - 1))
                    return ps

                pslo = {}

                def emit_lo(b):
                    # lo chain accumulates in PSUM, then spills to SBUF so the
                    # bank frees immediately (keeps the lo-ahead window deep)
                    if Tlo[b] == 0:
                        return
                    ps = emit_chain(b, "lo", psl_pool, "pl")
                    sp = lop_pool.tile([128, HC], F16, tag="lop",
                                       name=f"lop{l}_{b}")
                    nc.vector.tensor_copy(sp[:], ps[:])
                    pslo[b] = sp

                def emit_full(b):
                    """Baseline path: one PSUM chain over lo+hi tiles."""
                    ntiles = Tlo[b] + Thi[b]
                    ps = None
                    if ntiles > 0:
                        ps = psh_pool.tile([128, HC], F32, tag="ph",
                                           name=f"ph{l}_{b}")
                        i = 0
                        for sname in ("lo", "hi"):
                            s = streams[sname]
                            for j in range(s["nt"][b]):
                                t = s["t0"][b] + j
                                k = t // TC
                                g, ind = fetch(s, sname, k)
                                jj = t - k * TC
                                nc.tensor.matmul(
                                    ps[:], ind[:, jj * 128:(jj + 1) * 128],
                                    g[:, jj * 128:jj * 128 + HC],
                                    start=(i == 0), stop=(i == ntiles - 1))
                                i += 1
                    epilogue(b, ps, None)

                def emit_hi_epi(b):
                    ps = None
                    if Thi[b] > 0:
                        ps = emit_chain(b, "hi", psh_pool, "ph")
                    epilogue(b, ps, pslo.pop(b, None))

                def epilogue(b, ps, sp):
                    tmp = tmp_pool.tile([128, HC], F32, tag="tmp")
                    tmp0 = tmp_pool.tile([128, HC], F32, tag="tmp0")
                    hps = hp_own[:, b * 128:b * 128 + HC]
                    first = ps[:] if ps is not None else (
                        sp[:] if sp is not None else None)
                    if first is not None:
                        # S + selfw*h_own  (self-edges folded out of the gather)
                        nc.vector.scalar_tensor_tensor(
                            tmp0[:], hps, C["selfw"][:, b:b + 1], first,
                            Alu.mult, Alu.add)
                    else:
                        nc.vector.tensor_scalar(
                            tmp0[:], hps, C["selfw"][:, b:b + 1], None, Alu.mult)
                    if ps is not None and sp is not None:
                        tmpa = tmp_pool.tile([128, HC], F32, tag="tmpa")
                        nc.vector.tensor_tensor(tmpa[:], tmp0[:], sp[:], Alu.add)
                        tmp0 = tmpa
                    nc.vector.scalar_tensor_tensor(
                        tmp[:], tmp0[:], dinv_sb[:, b:b + 1], b_sb[l][:],
                        Alu.mult, Alu.add)
                    if premult:
                        nc.scalar.activation(
                            out_big[:, b * 128:b * 128 + HC], tmp[:], Act.Relu,
                            scale=dinv_sb[:, b:b + 1])
                    else:
                        nc.scalar.activation(
                            out_big[:, b * 128:b * 128 + HC], tmp[:], Act.Relu)
                    # transposed copy for the next dense / MLP (feature-major)
                    ptr = ptr_pool.tile([128, 128], F16, tag="ptr", space="PSUM")
                    nc.tensor.transpose(
                        ptr[:], out_big[:, b * 128:(b + 1) * 128], C["ident"][:])
                    nc.vector.tensor_copy(zt[:, b * 128:(b + 1) * 128], ptr[:])
                    on_block(b, zt)

                # lo-ahead window: the first W blocks' lo-stream gathers are
                # emitted before any hi-dependent work so gpsimd stays busy
                # while the hi-half AllGather is still in flight; the rest of
                # the stage uses the baseline single-chain path
                for b in range(min(W, NB)):
                    emit_lo(b)
                for b in range(NB):
                    if b < W:
                        emit_hi_epi(b)
                    else:
                        emit_full(b)
                return out_big, zt

            # layer 1 dense (x streamed per block from DRAM)
            hp1 = big_alloc("hpd0")
            for t in range(NB):
                dense_block(0, x_src, hp1, t, scale_dinv=True)

            hp2, ob1 = make_dense_consumer(1, HC)
            z2, zt2 = gather_stage(0, True, hp1, ob1)
            hp3, ob2 = make_dense_consumer(2, HC)
            z3, zt3 = gather_stage(1, True, hp2, ob2)
            y3, y3t = gather_stage(2, False, hp3, mlp_consumer)

    nc.compile()
    return nc


_CACHE = {}


def kernel(**inputs):
    structure, in_maps = _preprocess(inputs)
    key = (tuple(structure["Tlo"]), tuple(structure["Thi"]))
    if key not in _CACHE:
        _CACHE[key] = _build(structure)
    nc = _CACHE[key]
    res = run_bass_kernel_spmd(nc, in_maps, core_ids=list(range(NCORES)))
    out = np.empty((N, OUT), np.float32)
    for c in range(NCORES):
        out[c * NPER:(c + 1) * NPER] = res.results[c]["out"].T[:NPER]
    return out



# revision 32
# speedup vs baseline: 1.1890x; 1.0390x over previous
"""GCN message-passing kernel for 8 TRN2 NeuronCores.

Strategy (graph/data parallel, dst-sharded):
  - Nodes sharded 6250/core. Per layer: each core computes its shard of the
    gather table H' (dense matmul + epilogue), an AllGather builds the full
    50000x128 fp16 table in HBM.
  - Edges (incl. self-loops) sorted by dst, grouped into 128-wide dst blocks.
    Per 128-edge tile: dma_gather fetches H'[src] rows (256B fp16), DVE
    is_equal builds the one-hot [edge, dst_local] tile, PE accumulates
    Onehot^T @ G into the block's PSUM (segment sum).
  - GCN norm dinv[src]*dinv[dst] is separable: src side folded into the
    table rows, dst side into the block epilogue. BN folded into W and B.
  - Epilogues: dense1: tbl = dinv*(x@W1A); gather1/2: z = dinv*relu(S*dinv+B)
    (the outer dinv pre-applies the next table's src factor); dense2/3 are
    then plain copies; gather3: y3 = relu(S*dinv+B) feeds the MLP head.
  - 4-layer MLP runs feature-major per core; output [8, 6272] f32 is
    transposed/trimmed on the host.
"""

import sys

sys.path.insert(0, "/opt/trn_rl_repo")

import numpy as np

import concourse.bacc as bacc
import concourse.mybir as mybir
import concourse.tile as tile
from concourse.bass_utils import run_bass_kernel_spmd

# Problem constants
N, E, IN, HC = 50000, 800000, 64, 96
FC1, FC2, FC3, OUT = 256, 128, 64, 8
EPS = 1e-5
NCORES = 8
NPER = N // NCORES          # 6250 nodes per core
BW = 128                    # dst-block width
NB = (NPER + BW - 1) // BW  # 49 blocks
NPAD = NB * BW              # 6272
TC = 8                      # tiles per gather chunk (1024 idxs; >1024 hangs HW)
W = 18                      # lo-ahead window (blocks) to hide the hi AllGather


def _set_dims(n, e, split=None, tc=None):
    """Test hook: shrink the problem (keeps feature dims)."""
    global N, E, NPER, NB, NPAD, TC
    N, E = n, e
    NPER = N // NCORES
    NB = (NPER + BW - 1) // BW
    NPAD = NB * BW
    if tc is not None:
        TC = tc

def _halves():
    """Split the NB blocks into two AllGather halves (lo/hi tables).
    Asymmetric: a smaller hi half shortens the boundary-critical hi
    AllGather while the bigger lo half deepens the lo-ahead runway."""
    bl = min(29, NB - 1) if NB > 1 else 1
    return bl, NB - bl  # blocks in lo, hi


F16 = mybir.dt.float16
F32 = mybir.dt.float32
I16 = mybir.dt.int16
F8 = mybir.dt.float8e4
F8NP = mybir.dt.np(F8)  # ml_dtypes.float8_e4m3
FP8_ONE = 1.0

Alu = mybir.AluOpType
Act = mybir.ActivationFunctionType


def _wrap_idx(idx_stream):
    """int16 idx stream -> [128, len/16] wrapped layout (pos i -> [i%16, i//16]),
    replicated 8x across partition groups."""
    n = idx_stream.shape[0]
    assert n % 16 == 0
    a = idx_stream.reshape(n // 16, 16).T.astype(np.int16)
    return np.ascontiguousarray(np.tile(a, (8, 1)))


def _preprocess(inputs):
    """Host-side graph preprocessing. Returns (structure, per-core in_maps)."""
    x = np.asarray(inputs["x"], np.float32)
    edge_index = np.asarray(inputs["edge_index"], np.int64)
    src = np.concatenate([edge_index[0], np.arange(N, dtype=np.int64)])
    dst = np.concatenate([edge_index[1], np.arange(N, dtype=np.int64)])
    deg = np.bincount(dst, minlength=N).astype(np.float32)
    dinv = (1.0 / np.sqrt(deg)).astype(np.float32)

    order = np.argsort(dst, kind="stable")
    src_s = src[order]
    dst_s = dst[order]

    # self-edges (src==dst, incl. the synthetic self-loops) are folded into
    # the block epilogue as selfw[d] * h'_d instead of being gathered
    selfw_all = np.zeros(N, dtype=np.float32)
    self_m = src_s == dst_s
    np.add.at(selfw_all, dst_s[self_m], 1.0)
    src_s = src_s[~self_m]
    dst_s = dst_s[~self_m]

    # two-table AllGather layout: source node (c, r) lives in the lo table at
    # c*RLO + r (r < RLO) or the hi table at c*RHI + (r - RLO)
    BLO, BHI = _halves()
    RLO, RHI = BLO * 128, BHI * 128

    core_edge_start = np.searchsorted(dst_s, np.arange(0, N + 1, NPER))
    per_core = []
    nlo = np.zeros((NCORES, NB), dtype=np.int64)
    nhi = np.zeros((NCORES, NB), dtype=np.int64)
    for c in range(NCORES):
        s0, s1 = core_edge_start[c], core_edge_start[c + 1]
        sc, sr = src_s[s0:s1] // NPER, src_s[s0:s1] % NPER
        lo = sr < RLO
        cs = np.where(lo, sc * RLO + sr, sc * RHI + (sr - RLO))
        cd = dst_s[s0:s1] - c * NPER
        so = np.argsort(cs, kind="stable")  # src-sorted for HBM locality
        cs, cd, lo = cs[so], cd[so], lo[so]
        blk = cd // BW
        lists = []
        for b in range(NB):
            m = blk == b
            mlo = m & lo
            mhi = m & ~lo
            lists.append((cs[mlo], cd[mlo], cs[mhi], cd[mhi]))
            nlo[c, b] = int(mlo.sum())
            nhi[c, b] = int(mhi.sum())
        per_core.append(lists)

    Tlo = np.ceil(nlo.max(axis=0) / 128).astype(int)
    Thi = np.ceil(nhi.max(axis=0) / 128).astype(int)
    TLO, THI = int(Tlo.sum()), int(Thi.sum())
    lo_t0 = np.concatenate([[0], np.cumsum(Tlo)])[:-1]
    hi_t0 = np.concatenate([[0], np.cumsum(Thi)])[:-1]

    structure = dict(Tlo=Tlo.tolist(), Thi=Thi.tolist(), TLO=TLO, THI=THI,
                     lo_t0=lo_t0.tolist(), hi_t0=hi_t0.tolist())

    k = 1.0 / np.sqrt(1.0 + EPS)

    def fold(w, b, g, be):
        A = (np.asarray(g, np.float32) * k)
        Wp = (np.asarray(w, np.float32) * A[None, :]).astype(np.float16)
        B = (np.asarray(b, np.float32) * A + np.asarray(be, np.float32))
        return Wp, np.tile(B[None, :].astype(np.float32), (128, 1))

    w1p, b1rep = fold(inputs["w1"], inputs["b1"], inputs["g1"], inputs["be1"])
    w2p, b2rep = fold(inputs["w2"], inputs["b2"], inputs["g2"], inputs["be2"])
    w3p, b3rep = fold(inputs["w3"], inputs["b3"], inputs["g3"], inputs["be3"])

    lw1 = np.asarray(inputs["lw1"], np.float32).astype(np.float16)
    lw2 = np.asarray(inputs["lw2"], np.float32).astype(np.float16)
    lw3 = np.asarray(inputs["lw3"], np.float32).astype(np.float16)
    lw4 = np.asarray(inputs["lw4"], np.float32).astype(np.float16)

    shared = {
        "w1p": w1p, "w2p": w2p, "w3p": w3p,
        "b1rep": b1rep, "b2rep": b2rep, "b3rep": b3rep,
        "lw1a": np.ascontiguousarray(lw1[:, :128]),
        "lw1b": np.ascontiguousarray(lw1[:, 128:]),
        "lw2a": np.ascontiguousarray(lw2[:128, :]),
        "lw2b": np.ascontiguousarray(lw2[128:, :]),
        "lw3": lw3, "lw4": lw4,
        "lb1a": np.asarray(inputs["lb1"], np.float32)[:128, None].copy(),
        "lb1b": np.asarray(inputs["lb1"], np.float32)[128:, None].copy(),
        "lb2": np.asarray(inputs["lb2"], np.float32)[:, None].copy(),
        "lb3": np.asarray(inputs["lb3"], np.float32)[:, None].copy(),
        "lb4": np.asarray(inputs["lb4"], np.float32)[:, None].copy(),
        "ident": np.eye(128, dtype=np.float16),
        "iota": np.tile(np.arange(128, dtype=np.float16)[None, :], (128, 1)),
    }

    in_maps = []
    for c in range(NCORES):
        idx_lo = np.zeros(max(TLO, 1) * 128, dtype=np.int16)
        idx_hi = np.zeros(max(THI, 1) * 128, dtype=np.int16)
        # dst-local id streams: did[p, t] = local dst (0..127) of edge (t, p);
        # sentinel 300 for pad slots (never matches the 0..127 iota row)
        did_lo = np.full((128, max(TLO, 1)), 300.0, dtype=np.float16)
        did_hi = np.full((128, max(THI, 1)), 300.0, dtype=np.float16)
        for b in range(NB):
            cs_lo, cd_lo, cs_hi, cd_hi = per_core[c][b]
            o = lo_t0[b] * 128
            idx_lo[o:o + len(cs_lo)] = cs_lo.astype(np.int16)
            pos = o + np.arange(len(cd_lo))
            did_lo[pos % 128, pos // 128] = (cd_lo - b * BW).astype(np.float16)
            o = hi_t0[b] * 128
            idx_hi[o:o + len(cs_hi)] = cs_hi.astype(np.int16)
            pos = o + np.arange(len(cd_hi))
            did_hi[pos % 128, pos // 128] = (cd_hi - b * BW).astype(np.float16)

        dv = np.ones(NPAD, dtype=np.float32)
        dv[:NPER] = dinv[c * NPER:(c + 1) * NPER]
        dinv_loc = np.ascontiguousarray(dv.reshape(NB, 128).T)

        sw = np.zeros(NPAD, dtype=np.float32)
        sw[:NPER] = selfw_all[c * NPER:(c + 1) * NPER]
        selfw_loc = np.ascontiguousarray(sw.reshape(NB, 128).T)

        xT = np.zeros((IN, NPAD), dtype=np.float16)
        xT[:, :NPER] = x[c * NPER:(c + 1) * NPER].T.astype(np.float16)

        m = dict(shared)
        m.update({
            "xT": xT,
            "dinv": dinv_loc,
            "selfw": selfw_loc,
            "idxlo": _wrap_idx(idx_lo),
            "idxhi": _wrap_idx(idx_hi),
            "dlo": did_lo,
            "dhi": did_hi,
        })
        in_maps.append(m)

    return structure, in_maps


def _build(structure):
    """Build the SPMD Bass graph (shared by all 8 cores)."""
    Tlo, Thi = structure["Tlo"], structure["Thi"]
    TLO, THI = structure["TLO"], structure["THI"]
    lo_t0, hi_t0 = structure["lo_t0"], structure["hi_t0"]
    TLOp, THIp = max(TLO, 1), max(THI, 1)
    cores = list(range(NCORES))

    nc = bacc.Bacc("TRN2", target_bir_lowering=False, debug=False,
                   num_devices=NCORES, num_swdge_queues=4,
                   dynamic_dma_scratch_size=57344)

    P = {}
    def par(name, shape, dtype, out=False):
        P[name] = nc.declare_dram_parameter(name, shape, dtype, isOutput=out)
        return P[name]

    par("xT", [IN, NPAD], F16)
    par("w1p", [IN, HC], F16); par("w2p", [HC, HC], F16); par("w3p", [HC, HC], F16)
    par("b1rep", [128, HC], F32); par("b2rep", [128, HC], F32); par("b3rep", [128, HC], F32)
    par("dinv", [128, NB], F32)
    par("selfw", [128, NB], F32)
    par("ident", [128, 128], F16)
    par("iota", [128, 128], F16)
    par("idxlo", [128, 8 * TLOp], I16); par("idxhi", [128, 8 * THIp], I16)
    par("dlo", [128, TLOp], F16); par("dhi", [128, THIp], F16)
    par("lw1a", [HC, 128], F16); par("lw1b", [HC, FC1 - 128], F16)
    par("lw2a", [128, FC2], F16); par("lw2b", [FC1 - 128, FC2], F16)
    par("lw3", [FC2, FC3], F16); par("lw4", [FC3, OUT], F16)
    par("lb1a", [128, 1], F32); par("lb1b", [FC1 - 128, 1], F32)
    par("lb2", [FC2, 1], F32); par("lb3", [FC3, 1], F32); par("lb4", [OUT, 1], F32)
    par("out", [OUT, NPAD], F32, out=True)

    with tile.TileContext(nc) as tc:
        with (
            tc.tile_pool(name="const", bufs=1) as cpool,
            tc.tile_pool(name="dram", bufs=1, space="DRAM") as dpool,
            tc.tile_pool(name="hp", bufs=3) as hp_pool,
            tc.tile_pool(name="zt", bufs=2) as zt_pool,
            tc.tile_pool(name="xt", bufs=3) as xt_pool,
            tc.tile_pool(name="glo", bufs=4) as glo_pool,
            tc.tile_pool(name="ghi", bufs=4) as ghi_pool,
            tc.tile_pool(name="ind", bufs=3) as ind_pool,
            tc.tile_pool(name="tmp", bufs=4) as tmp_pool,
            tc.tile_pool(name="lop", bufs=22) as lop_pool,
            tc.tile_pool(name="mz", bufs=2) as mz_pool,
            tc.tile_pool(name="osb", bufs=2) as osb_pool,
            tc.tile_pool(name="psd", bufs=1, space="PSUM") as psd_pool,
            tc.tile_pool(name="psl", bufs=2, space="PSUM") as psl_pool,
            tc.tile_pool(name="psh", bufs=2, space="PSUM") as psh_pool,
            tc.tile_pool(name="ptr", bufs=1, space="PSUM") as ptr_pool,
        ):
            C = {}
            for name, p in P.items():
                if name in ("out", "xT"):  # xT is streamed per dense block
                    continue
                t = cpool.tile(list(p.shape), p.dtype, name=f"c_{name}")
                nc.sync.dma_start(t[:], p[:])
                C[name] = t

            BLO, BHI = _halves()
            RLO, RHI = BLO * 128, BHI * 128
            agin = [[dpool.tile([RLO, 128], F16, name=f"aginlo{l}"),
                     dpool.tile([RHI, 128], F16, name=f"aginhi{l}")]
                    for l in range(3)]
            hlo = [dpool.tile([NCORES * RLO, 128], F16, addr_space="Shared",
                              name=f"hlo{l}") for l in range(3)]
            hhi = [dpool.tile([NCORES * RHI, 128], F16, addr_space="Shared",
                              name=f"hhi{l}") for l in range(3)]

            w_sb = [C["w1p"], C["w2p"], C["w3p"]]
            b_sb = [C["b1rep"], C["b2rep"], C["b3rep"]]
            dinv_sb = C["dinv"]

            def big_alloc(name):
                """[128, NPAD] fp16 big tile with pad cols (96:128) zeroed."""
                t = hp_pool.tile([128, NPAD], F16, tag="hp", name=name)
                nc.vector.memset(
                    t[:].rearrange("p (t f) -> p t f", f=128)[:, :, HC:128], 0.0)
                return t

            def emit_half_ag(l, hp, b):
                """After block b (a half boundary), ship that half + AllGather."""
                if b == BLO - 1:
                    k, b0, nb, tbl = 0, 0, BLO, hlo[l]
                elif b == NB - 1:
                    k, b0, nb, tbl = 1, BLO, BHI, hhi[l]
                else:
                    return
                ag = agin[l][k]
                nc.sync.dma_start(
                    ag[:].rearrange("(t p) f -> p t f", p=128),
                    hp[:, b0 * 128:(b0 + nb) * 128].rearrange(
                        "p (t f) -> p t f", f=128))
                nc.gpsimd.collective_compute(
                    "AllGather", Alu.bypass, replica_groups=[cores],
                    ins=[ag[:]], outs=[tbl[:]])

            def dense_block(l, src_ap, hp, t, scale_dinv):
                ps = psd_pool.tile([128, HC], F32, tag="psd",
                                   name=f"psd{l}_{t}")
                nc.tensor.matmul(ps[:], src_ap(t), w_sb[l][:],
                                 start=True, stop=True)
                if scale_dinv:
                    nc.vector.tensor_scalar(
                        hp[:, t * 128:t * 128 + HC], ps[:],
                        dinv_sb[:, t:t + 1], None, Alu.mult)
                else:
                    nc.vector.tensor_copy(hp[:, t * 128:t * 128 + HC], ps[:])
                emit_half_ag(l, hp, t)

            def x_src(t):
                xt = xt_pool.tile([IN, 128], F16, tag="xt", name=f"xt{t}")
                nc.sync.dma_start(xt[:], P["xT"][:, t * 128:(t + 1) * 128])
                return xt[:]

            def make_dense_consumer(l1, fin):
                hp = big_alloc(f"hpd{l1}")
                def on_block(b, zt):
                    dense_block(l1, lambda t: zt[0:fin, t * 128:(t + 1) * 128],
                                hp, b, scale_dinv=False)
                return hp, on_block

            # MLP head (feature-major), consumes y3t incrementally
            out_mlp_state = {}
            def mlp_chunk(r0, cw, y3t):
                rs = slice(r0, r0 + cw)
                p1a = psd_pool.tile([128, 512], F32, tag="mps", bufs=2)
                nc.tensor.matmul(p1a[:, 0:cw], C["lw1a"][:], y3t[0:HC, rs],
                                 start=True, stop=True)
                z1a = mz_pool.tile([128, 512], F16, tag="z1a")
                nc.scalar.activation(z1a[:, 0:cw], p1a[:, 0:cw], Act.Relu,
                                     bias=C["lb1a"][:])
                p1b = psd_pool.tile([128, 512], F32, tag="mps", bufs=2)
                nc.tensor.matmul(p1b[:, 0:cw], C["lw1b"][:], y3t[0:HC, rs],
                                 start=True, stop=True)
                z1b = mz_pool.tile([128, 512], F16, tag="z1b")
                nc.scalar.activation(z1b[:, 0:cw], p1b[:, 0:cw], Act.Relu,
                                     bias=C["lb1b"][:])
                p2 = psd_pool.tile([128, 512], F32, tag="mps", bufs=2)
                nc.tensor.matmul(p2[:, 0:cw], C["lw2a"][:], z1a[:, 0:cw],
                                 start=True, stop=False)
                nc.tensor.matmul(p2[:, 0:cw], C["lw2b"][:], z1b[:, 0:cw],
                                 start=False, stop=True)
                z2m = mz_pool.tile([128, 512], F16, tag="z2m")
                nc.scalar.activation(z2m[:, 0:cw], p2[:, 0:cw], Act.Relu,
                                     bias=C["lb2"][:])
                p3 = psd_pool.tile([FC3, 512], F32, tag="mps", bufs=2)
                nc.tensor.matmul(p3[:, 0:cw], C["lw3"][:], z2m[:, 0:cw],
                                 start=True, stop=True)
                z3m = mz_pool.tile([FC3, 512], F16, tag="z3m")
                nc.scalar.activation(z3m[:, 0:cw], p3[:, 0:cw], Act.Relu,
                                     bias=C["lb3"][:])
                p4 = psd_pool.tile([OUT, 512], F32, tag="mps", bufs=2)
                nc.tensor.matmul(p4[:, 0:cw], C["lw4"][:], z3m[:, 0:cw],
                                 start=True, stop=True)
                ob = osb_pool.tile([OUT, 512], F32, tag="ob")
                nc.vector.tensor_scalar(ob[:, 0:cw], p4[:, 0:cw], C["lb4"][:],
                                        None, Alu.add)
                nc.sync.dma_start(P["out"][:, rs], ob[:, 0:cw])

            def mlp_consumer(b, y3t):
                # emit an MLP chunk whenever 4 more blocks (512 cols) are ready
                r0 = (b // 4) * 512
                if b == NB - 1:
                    while r0 < NPAD:
                        mlp_chunk(r0, min(512, NPAD - r0), y3t)
                        r0 += 512
                        out_mlp_state["r0"] = r0
                elif b % 4 == 3:
                    mlp_chunk(r0, 512, y3t)
                    out_mlp_state["r0"] = r0 + 512

            qctr = [0]

            def gather_stage(l, premult, hp_own, on_block):
                """Aggregate edges against tables l; returns (out_big, zt)."""
                out_big = big_alloc(f"agg{l}")
                zt = zt_pool.tile([128, NPAD], F16, tag="zt", name=f"zt{l}")
                streams = {
                    "lo": dict(T=TLO, t0=lo_t0, nt=Tlo, did=C["dlo"],
                               idx=C["idxlo"], src=hlo[l][:],
                               pool=glo_pool, cache={}),
                    "hi": dict(T=THI, t0=hi_t0, nt=Thi, did=C["dhi"],
                               idx=C["idxhi"], src=hhi[l][:],
                               pool=ghi_pool, cache={}),
                }

                def fetch(s, sname, k):
                    if k in s["cache"]:
                        return s["cache"][k]
                    t0 = k * TC
                    ct = min(TC, s["T"] - t0)
                    nidx = ct * 128
                    g = s["pool"].tile([128, TC * 128], F16, tag=f"g{sname}",
                                       name=f"g{sname}_{l}_{k}")
                    nc.gpsimd.dma_gather(
                        g[:, 0:nidx].rearrange("p (t e) -> p t e", e=128),
                        s["src"],
                        s["idx"][:, t0 * 8:(t0 + ct) * 8],
                        nidx, nidx, 128,
                        queue_num=qctr[0] % 4)
                    qctr[0] += 1
                    ind = ind_pool.tile([128, TC * 128], F8, tag=f"i{sname}",
                                        name=f"i{sname}_{l}_{k}")
                    # on-chip one-hot: ind[p, t, d] = (did[p, t0+t] == d)
                    nc.vector.tensor_tensor(
                        ind[:, 0:nidx].rearrange("p (t e) -> p t e", e=128),
                        C["iota"][:].rearrange("p (t e) -> p t e", t=1)
                            .to_broadcast([128, ct, 128]),
                        s["did"][:, t0:t0 + ct].to_broadcast([128, ct, 128]),
                        Alu.is_equal)
                    s["cache"][k] = (g, ind)
                    return s["cache"][k]

                def emit_chain(b, sname, pool, ptag):
                    """Accumulate stream `sname`'s tiles of block b in PSUM."""
                    s = streams[sname]
                    nt = s["nt"][b]
                    ps = pool.tile([128, HC], F32, tag=ptag,
                                   name=f"{ptag}{l}_{b}")
                    for i in range(nt):
                        t = s["t0"][b] + i
                        k = t // TC
                        g, ind = fetch(s, sname, k)
                        jj = t - k * TC
                        nc.tensor.matmul(
                            ps[:], ind[:, jj * 128:(jj + 1) * 128],
                            g[:, jj * 128:jj * 128 + HC],
                            start=(i == 0), stop=(i == nt - 1))
                    return ps

                pslo = {}

                def emit_lo(b):
                    # lo chain accumulates in PSUM, then spills to SBUF so the
                    # bank frees immediately (keeps the lo-ahead window deep)
                    if Tlo[b] == 0:
                        return
                    ps = emit_chain(b, "lo", psl_pool, "pl")
                    sp = lop_pool.tile([128, HC], F16, tag="lop",
                                       name=f"lop{l}_{b}")
                    nc.vector.tensor_copy(sp[:], ps[:])
                    pslo[b] = sp

                def emit_full(b):
                    """Baseline path: one PSUM chain over lo+hi tiles."""
                    ntiles = Tlo[b] + Thi[b]
                    ps = None
                    if ntiles > 0:
                        ps = psh_pool.tile([128, HC], F32, tag="ph",
                                           name=f"ph{l}_{b}")
                        i = 0
                        for sname in ("lo", "hi"):
                            s = streams[sname]
                            for j in range(s["nt"][b]):
                                t = s["t0"][b] + j
                                k = t // TC
                                g, ind = fetch(s, sname, k)
                                jj = t - k * TC
                                nc.tensor.matmul(
                                    ps[:], ind[:, jj * 128:(jj + 1) * 128],
                                    g[:, jj * 128:jj * 128 + HC],
                                    start=(i == 0), stop=(i == ntiles - 1))
                                i += 1
                    epilogue(b, ps, None)

                def emit_hi_epi(b):
                    ps = None
                    if Thi[b] > 0:
                        ps = emit_chain(b, "hi", psh_pool, "ph")
                    epilogue(b, ps, pslo.pop(b, None))

                def epilogue(b, ps, sp):
                    tmp = tmp_pool.tile([128, HC], F32, tag="tmp")
                    tmp0 = tmp_pool.tile([128, HC], F32, tag="tmp0")
                    hps = hp_own[:, b * 128:b * 128 + HC]
                    first = ps[:] if ps is not None else (
                        sp[:] if sp is not None else None)
                    if first is not None:
                        # S + selfw*h_own  (self-edges folded out of the gather)
                        nc.vector.scalar_tensor_tensor(
                            tmp0[:], hps, C["selfw"][:, b:b + 1], first,
                            Alu.mult, Alu.add)
                    else:
                        nc.vector.tensor_scalar(
                            tmp0[:], hps, C["selfw"][:, b:b + 1], None, Alu.mult)
                    if ps is not None and sp is not None:
                        tmpa = tmp_pool.tile([128, HC], F32, tag="tmpa")
                        nc.vector.tensor_tensor(tmpa[:], tmp0[:], sp[:], Alu.add)
                        tmp0 = tmpa
                    nc.vector.scalar_tensor_tensor(
                        tmp[:], tmp0[:], dinv_sb[:, b:b + 1], b_sb[l][:],
                        Alu.mult, Alu.add)
                    if premult:
                        nc.scalar.activation(
                            out_big[:, b * 128:b * 128 + HC], tmp[:], Act.Relu,
                            scale=dinv_sb[:, b:b + 1])
                    else:
                        nc.scalar.activation(
                            out_big[:, b * 128:b * 128 + HC], tmp[:], Act.Relu)
                    # transposed copy for the next dense / MLP (feature-major)
                    ptr = ptr_pool.tile([128, 128], F16, tag="ptr", space="PSUM")
                    nc.tensor.transpose(
                        ptr[:], out_big[:, b * 128:(b + 1) * 128], C["ident"][:])
                    nc.vector.tensor_copy(zt[:, b * 128:(b + 1) * 128], ptr[:])
                    on_block(b, zt)

                # lo-ahead window: the first W blocks' lo-stream gathers are
                # emitted before any hi-dependent work so gpsimd stays busy
                # while the hi-half AllGather is still in flight; the rest of
                # the stage uses the baseline single-chain path
                for b in range(min(W, NB)):
                    emit_lo(b)
                for b in range(NB):
                    if b < W:
                        emit_hi_epi(b)
                    else:
                        emit_full(b)
                return out_big, zt

            # layer 1 dense (x streamed per block from DRAM)
            hp1 = big_alloc("hpd0")
            for t in range(NB):
                dense_block(0, x_src, hp1, t, scale_dinv=True)

            hp2, ob1 = make_dense_consumer(1, HC)
            z2, zt2 = gather_stage(0, True, hp1, ob1)
            hp3, ob2 = make_dense_consumer(2, HC)
            z3, zt3 = gather_stage(1, True, hp2, ob2)
            y3, y3t = gather_stage(2, False, hp3, mlp_consumer)

    nc.compile()
    return nc


_CACHE = {}


def kernel(**inputs):
    structure, in_maps = _preprocess(inputs)
    key = (tuple(structure["Tlo"]), tuple(structure["Thi"]))
    if key not in _CACHE:
        _CACHE[key] = _build(structure)
    nc = _CACHE[key]
    res = run_bass_kernel_spmd(nc, in_maps, core_ids=list(range(NCORES)))
    out = np.empty((N, OUT), np.float32)
    for c in range(NCORES):
        out[c * NPER:(c + 1) * NPER] = res.results[c]["out"].T[:NPER]
    return out

